# revision 22
# baseline (speedup 1.0000x reference)
"""DiagSSMBlock Trainium2 kernel.

Math (matches the reference exactly):
    s = b_mat.T @ x_seq.T                  # (H, T)
    y[h, t] = a[h] * y[h, t-1] + s[h, t]   # first-order IIR scan along t
    out = y.T                              # (T, H)

Sharding: a 2 (H) x 4 (T) grid over 8 cores. Each core computes a
(1024 channels x 1024 timesteps) output block: a (2048x1024)^T @
(2048x1024) matmul accumulated over K=2048 in PSUM, then the
per-channel IIR scan via the Vector engine's tensor_tensor_scan.

Time-sharding needs no cross-core communication: |a| <= sqrt(2/2048)
~ 0.031, so the scan state decays below fp32 noise within a few steps.
Each core's scan is seeded with a carry computed on the host from a
16-column warm-up strip (a^17 ~ 1e-25 of history is dropped -- exactly
zero in fp32). The strip matmul is 0.1% of the device FLOPs.

Matmul operands are bf16 (PE streams bf16 at the same 1 column/cycle
as float32r but with half the HBM traffic; rel err ~2e-3 vs the 2e-2
budget). Accumulation stays fp32 in PSUM.

Schedule: the 8 output m-tiles are processed in groups of (3, 3, 2).
Group 0 runs k-major, chasing the x DMA stream; its 20.7us of PE work
covers the ~17us the full x stream needs at ~358 GB/s, so the PE --
not the DMA -- paces the kernel from the first tile onward. Groups 1-2
run m-major on resident data. The final m-tile finishes in fine chunks
(256-col scans on separate PSUM tiles, 3-piece output DMA) to minimise
the kernel tail.

DMA strategy: every transfer is [128 partitions x contiguous bytes]
from a host-pre-swizzled DRAM image, so each dma_start is 128 large
descriptors. HWDGE descriptor generation costs ~0.6us *serialized* per
dma_start on the issuing engine, so inputs are ~21 dma_starts on the
Sync ring instead of 130 row-tile transfers. Outputs go on the Scalar
HWDGE ring. The PE is warmed with junk matmuls on a memset tile while
the first x k-tile streams in, releasing the HAM clock gate (1.2 ->
2.4 GHz after ~3.4us of sustained activity) before the real stream.
"""

import sys

import ml_dtypes
import numpy as np

_REPO = "/opt/trn_rl_repo"
if _REPO not in sys.path:
    sys.path.insert(0, _REPO)

import concourse.bass as bass
import concourse.mybir as mybir
from concourse import bacc
from concourse.bass_utils import run_bass_kernel_spmd
from concourse.tile import TileContext

T = 4096
H = 2048
NCORES = 8
HG = 2           # h groups
TG = 4           # t groups
HSH = H // HG    # 1024 channels per core
TSH = T // TG    # 1024 timesteps per core
WARM = 16        # host-side scan warm-up columns per t boundary
P = 128
KT = H // P      # 16 k-tiles
MT = HSH // P    # 8 m-tiles
GROUPS = ((0, 1, 2), (3, 4, 5), (6, 7))
G0W = len(GROUPS[0]) * P      # 384 b cols carried with each x k-tile
XW = TSH + G0W                # 1408 cols per x k-tile
B1W = KT * len(GROUPS[1]) * P  # group-1 b block cols
B2W = KT * len(GROUPS[2]) * P  # group-2 b block cols
NCH = 512
CHUNKS = ((0, NCH), (NCH, NCH))
WU = 1           # PE warm-up matmuls (N=512) before the real stream
K0_PARTS = ((0, 256), (256, 512), (512, 1024))  # x k-tile 0 DMA/MM split

MM_DTYPE = mybir.dt.bfloat16
NP_MM = ml_dtypes.bfloat16

_nc_cache = {}


def build_nc(mm_dtype=MM_DTYPE):
    f32 = mybir.dt.float32
    nc = bacc.Bacc(None, target_bir_lowering=False)

    xkd = nc.declare_dram_parameter("xk", [P, KT * XW], mm_dtype, isOutput=False)
    bpd = nc.declare_dram_parameter("bp", [P, B1W + B2W], mm_dtype, isOutput=False)
    acvd = nc.declare_dram_parameter("acv", [P, 2 * MT], f32, isOutput=False)
    y = nc.declare_dram_parameter("y", [HSH, TSH], f32, isOutput=True)

    y_r = y.rearrange("(mo p) t -> p mo t", p=P)    # [128, 8, 1024]

    with TileContext(nc) as tc:
        with (
            tc.tile_pool(name="const", bufs=1) as cpool,
            tc.tile_pool(name="xp", bufs=KT) as xpool,
            tc.tile_pool(name="bpp", bufs=2) as bpool,
            tc.tile_pool(name="yp", bufs=MT - 2) as ypool,
            tc.tile_pool(name="ypl", bufs=2) as ylpool,
            tc.tile_pool(name="ps0", bufs=4, space="PSUM") as p0pool,
            tc.tile_pool(name="ps1", bufs=4, space="PSUM") as p1pool,
        ):
            acv_sb = cpool.tile([P, 2 * MT], f32)
            wt = cpool.tile([P, NCH], mm_dtype)
            # Warm-up operand on the GpSimd queue -- free earliest.
            nc.gpsimd.memset(wt[:], 0.0)

            # Input stream: x k-tile 0 is split into SEPARATE tiles (b
            # columns first, then three x pieces) so the first matmuls can
            # start as each small piece lands -- Tile tracks dependencies
            # at tile granularity, so pieces sharing a tile would all wait
            # for the last one.
            x0b = cpool.tile([P, G0W], mm_dtype)
            nc.sync.dma_start(out=x0b[:], in_=xkd[:, TSH:XW])
            x0parts = []
            for lo, hi in K0_PARTS:
                x0p = cpool.tile([P, hi - lo], mm_dtype, name=f"x0p{lo}")
                nc.sync.dma_start(out=x0p[:], in_=xkd[:, lo:hi])
                x0parts.append(x0p)
            x_tiles = [None]
            for k in range(1, KT):
                xk = xpool.tile([P, XW], mm_dtype, tag="x")
                nc.sync.dma_start(out=xk[:], in_=xkd[:, k * XW : (k + 1) * XW])
                if k == 3:
                    nc.sync.dma_start(out=acv_sb[:], in_=acvd[:])
                x_tiles.append(xk)
            bt1 = bpool.tile([P, B1W], mm_dtype, tag="b")
            nc.sync.dma_start(out=bt1[:], in_=bpd[:, 0:B1W])
            bt2 = bpool.tile([P, B2W], mm_dtype, tag="b")
            nc.sync.dma_start(out=bt2[:], in_=bpd[:, B1W : B1W + B2W])

            def b_tile(k, m):
                if m < 3:
                    if k == 0:
                        return x0b[:, m * P : (m + 1) * P]
                    return x_tiles[k][:, TSH + m * P : TSH + (m + 1) * P]
                if m < 6:
                    return bt1[:, k * G0W + (m - 3) * P : k * G0W + (m - 2) * P]
                return bt2[:, k * 2 * P + (m - 6) * P : k * 2 * P + (m - 5) * P]

            def k0_mms(ps_ap, m, lo, hi, stop):
                # k=0 contribution for output cols [lo,hi) from the split
                # tile-0 pieces; must come after the bank-clearing start=True
                # matmul of its accumulation group (order inside a PSUM
                # accumulation group is irrelevant).
                for pi, (plo, phi) in enumerate(K0_PARTS):
                    s, e = max(lo, plo), min(hi, phi)
                    if s >= e:
                        continue
                    nc.tensor.matmul(
                        ps_ap[:, s - lo : e - lo],
                        b_tile(0, m),
                        x0parts[pi][:, s - plo : e - plo],
                        start=False,
                        stop=(stop and e == hi),
                    )

            def scan(ym, m, c0, cw, data1, first):
                nc.vector.tensor_tensor_scan(
                    out=ym[:, c0 : c0 + cw],
                    data0=acv_sb[:, m : m + 1].broadcast_to((P, cw)),
                    data1=data1,
                    initial=(
                        acv_sb[:, MT + m : MT + m + 1]
                        if first
                        else ym[:, c0 - 1 : c0]
                    ),
                    op0=mybir.AluOpType.mult,
                    op1=mybir.AluOpType.add,
                )

            # PE warm-up: junk matmuls release the HAM clock gate while the
            # first x tile streams in.
            wps = p0pool.tile([P, NCH], f32, tag="ps0")
            for _ in range(WU):
                nc.tensor.matmul(wps[:], wt[:, 0:P], wt[:], start=True, stop=True)

            # ---- Group 0: k-major, chases the x DMA stream ----
            g0 = GROUPS[0]
            ps = {}
            for m in g0:
                ps[(m, 0)] = p0pool.tile([P, NCH], f32, tag="ps0", name=f"ps0_m{m}")
                ps[(m, 1)] = p1pool.tile([P, NCH], f32, tag="ps1", name=f"ps1_m{m}")
            for k in range(KT):
                if k == 0:
                    # Follow the split tile-0 arrival sub-chunk by sub-chunk.
                    # start=True clears the whole PSUM bank, so only the
                    # first write into each bank sets it; untouched regions
                    # have has_written=0 and get overwritten (not summed)
                    # by their own first matmul.
                    for pi, (lo, hi) in enumerate(K0_PARTS):
                        ci = 0 if lo < NCH else 1
                        for m in g0:
                            nc.tensor.matmul(
                                ps[(m, ci)][:, lo - ci * NCH : hi - ci * NCH],
                                b_tile(0, m),
                                x0parts[pi][:],
                                start=(lo % NCH == 0),
                                stop=False,
                            )
                    continue
                for m in g0:
                    for ci in range(2):
                        c0, cw = CHUNKS[ci]
                        nc.tensor.matmul(
                            ps[(m, ci)][:],
                            b_tile(k, m),
                            x_tiles[k][:, c0 : c0 + cw],
                            start=False,
                            stop=(k == KT - 1),
                        )
            for m in g0:
                ym = ypool.tile([P, TSH], f32, tag="y")
                for ci, (c0, cw) in enumerate(CHUNKS):
                    scan(ym, m, c0, cw, ps[(m, ci)][:], ci == 0)
                nc.scalar.dma_start(out=y_r[:, m, :], in_=ym[:])

            # ---- Group 1: m-major on resident data ----
            for m in GROUPS[1]:
                pm = (
                    p0pool.tile([P, NCH], f32, tag="ps0", name=f"ps0_m{m}"),
                    p1pool.tile([P, NCH], f32, tag="ps1", name=f"ps1_m{m}"),
                )
                for k in range(1, KT):
                    for ci, (c0, cw) in enumerate(CHUNKS):
                        nc.tensor.matmul(
                            pm[ci][:],
                            b_tile(k, m),
                            x_tiles[k][:, c0 : c0 + cw],
                            start=(k == 1),
                            stop=False,
                        )
                for ci, (c0, cw) in enumerate(CHUNKS):
                    k0_mms(pm[ci][:], m, c0, c0 + cw, True)
                ym = ypool.tile([P, TSH], f32, tag="y")
                for ci, (c0, cw) in enumerate(CHUNKS):
                    scan(ym, m, c0, cw, pm[ci][:], ci == 0)
                nc.scalar.dma_start(out=y_r[:, m, :], in_=ym[:])

            # ---- Group 2: m6 chunk-major, m7 fine-grained tail ----
            m = 6
            pm = (
                p0pool.tile([P, NCH], f32, tag="ps0", name="ps0_m6"),
                p1pool.tile([P, NCH], f32, tag="ps1", name="ps1_m6"),
            )
            ym = ylpool.tile([P, TSH], f32, tag="ylast")
            for ci, (c0, cw) in enumerate(CHUNKS):
                for k in range(1, KT):
                    nc.tensor.matmul(
                        pm[ci][:],
                        b_tile(k, m),
                        x_tiles[k][:, c0 : c0 + cw],
                        start=(k == 1),
                        stop=False,
                    )
                k0_mms(pm[ci][:], m, c0, c0 + cw, True)
                scan(ym, m, c0, cw, pm[ci][:], ci == 0)
            nc.scalar.dma_start(out=y_r[:, m, :], in_=ym[:])

            m = 7
            pA = p0pool.tile([P, NCH], f32, tag="ps0")
            pB1f = p1pool.tile([P, NCH], f32, tag="ps1")
            pB2f = p0pool.tile([P, NCH], f32, tag="ps0")
            pB1 = pB1f[:, 0:256]
            pB2 = pB2f[:, 0:256]
            ym = ylpool.tile([P, TSH], f32, tag="ylast")
            for k in range(1, KT):
                nc.tensor.matmul(
                    pA[:], b_tile(k, m), x_tiles[k][:, 0:NCH],
                    start=(k == 1), stop=False,
                )
            k0_mms(pA[:], m, 0, NCH, True)
            for k in range(1, KT):
                nc.tensor.matmul(
                    pB1[:], b_tile(k, m), x_tiles[k][:, NCH : NCH + 256],
                    start=(k == 1), stop=False,
                )
            k0_mms(pB1, m, NCH, NCH + 256, True)
            scan(ym, m, 0, 256, pA[:, 0:256], True)
            scan(ym, m, 256, 256, pA[:, 256:512], False)
            nc.scalar.dma_start(out=y_r[:, m, 0:512], in_=ym[:, 0:512])
            for k in range(1, KT):
                nc.tensor.matmul(
                    pB2[:], b_tile(k, m), x_tiles[k][:, NCH + 256 : TSH],
                    start=(k == 1), stop=False,
                )
            k0_mms(pB2, m, NCH + 256, TSH, True)
            scan(ym, m, 512, 256, pB1[:], False)
            nc.scalar.dma_start(out=y_r[:, m, 512:768], in_=ym[:, 512:768])
            scan(ym, m, 768, 256, pB2[:], False)
            nc.scalar.dma_start(out=y_r[:, m, 768:1024], in_=ym[:, 768:1024])
    nc.finalize()
    return nc


def make_in_maps(x_seq, a_diag, b_mat):
    x_seq = np.ascontiguousarray(np.asarray(x_seq, dtype=np.float32))
    a_diag = np.ascontiguousarray(np.asarray(a_diag, dtype=np.float32))
    b_mat = np.ascontiguousarray(np.asarray(b_mat, dtype=np.float32))
    assert x_seq.shape == (T, H) and a_diag.shape == (H,) and b_mat.shape == (H, H)

    xT = np.ascontiguousarray(x_seq.T)  # (H, T), K-major for the PE

    # Scan warm-up carries at each t-block boundary: scan a 16-column
    # strip of s = b^T x from zero state. History older than the strip
    # contributes < |a|^17 ~ 1e-25 relative -- exactly zero in fp32.
    carries = np.zeros((TG, H), dtype=np.float32)
    for tg in range(1, TG):
        strip = b_mat.T @ xT[:, tg * TSH - WARM : tg * TSH]  # (H, WARM)
        state = np.zeros(H, dtype=np.float32)
        for j in range(WARM):
            state = a_diag * state + strip[:, j]
        carries[tg] = state

    in_maps = []
    for c in range(NCORES):
        hg, tg = divmod(c, TG)
        hsl = slice(hg * HSH, (hg + 1) * HSH)
        xpart = xT[:, tg * TSH : (tg + 1) * TSH].reshape(KT, P, TSH)
        bcore = b_mat[:, hsl].reshape(KT, P, HSH)       # [k, p, 1024]
        xk = np.concatenate([xpart, bcore[:, :, 0:G0W]], axis=2)  # [k, p, 1408]
        xk = np.ascontiguousarray(
            xk.transpose(1, 0, 2).reshape(P, KT * XW).astype(NP_MM)
        )
        b1 = bcore[:, :, G0W : 2 * G0W].transpose(1, 0, 2).reshape(P, B1W)
        b2 = bcore[:, :, 2 * G0W : HSH].transpose(1, 0, 2).reshape(P, B2W)
        bp = np.ascontiguousarray(
            np.concatenate([b1, b2], axis=1).astype(NP_MM)
        )
        a_sw = a_diag[hsl].reshape(MT, P).T            # [128, 8]
        c_sw = carries[tg, hsl].reshape(MT, P).T       # [128, 8]
        acv = np.ascontiguousarray(
            np.concatenate([a_sw, c_sw], axis=1).astype(np.float32)
        )
        in_maps.append({"xk": xk, "bp": bp, "acv": acv})
    return in_maps


def run(in_maps, **kwargs):
    key = MM_DTYPE
    if key not in _nc_cache:
        _nc_cache[key] = build_nc(key)
    return run_bass_kernel_spmd(_nc_cache[key], in_maps, list(range(NCORES)), **kwargs)


def kernel(x_seq, a_diag, b_mat):
    res = run(make_in_maps(x_seq, a_diag, b_mat))
    yT = np.empty((H, T), dtype=np.float32)
    for c in range(NCORES):
        hg, tg = divmod(c, TG)
        yT[hg * HSH : (hg + 1) * HSH, tg * TSH : (tg + 1) * TSH] = res.results[c]["y"]
    return np.ascontiguousarray(yT.T)
